# revision 29
# baseline (speedup 1.0000x reference)
"""Per-row cosine-similarity loss (0.5 * cos(x1_row, x2_row)) on 8 TRN2 cores.

Pure data parallel: the batch dim (B=16384) is split into 8 shards of 2048
rows; each core computes its shard independently, no communication.

Per-core kernel (shard = [2048, 4096] f32 per tensor):
  - rows are tiled as row = p*16 + n  (p = SBUF partition, n = tile index),
    so each [128, 4096] tile is one ACT/DVE instruction and the final
    per-row result lands in a [128, 16] tile that stores with one DMA.
  - ACT (scalar engine): Square activation with accum_out -> per-row sum of
    squares for x1 and x2 (fused square+reduce, one pass per tensor).
  - DVE (vector engine): scalar_tensor_tensor(mult, mult, accum_out) ->
    per-row dot product (fused multiply+reduce, one pass).
  - Final [128, 16] math: cos = dot / (2*sqrt(sx)*sqrt(sy)) using
    sqrt(4*sx) = 2*sqrt(sx) to fold in the 0.5 factor.

The kernel is HBM-bound: 64 MiB input per core @ ~358 GB/s => ~187 us floor.
"""

import numpy as np

import concourse.bacc as bacc
import concourse.bass as bass
import concourse.tile as tile
from concourse import mybir
from concourse.bass_utils import run_bass_kernel_spmd

B, D = 16384, 4096
N_CORES = 8
B_SHARD = B // N_CORES  # 2048
P = 128
N_TILES = B_SHARD // P  # 16

_NC_CACHE = None
# kernel layout used by kernel(); host gather must match build_kernel()
SEQ_LAYOUT = False


def build_kernel(
    repeat: int = 1,
    bufs: int = 4,
    split_rings: bool = False,
    dma_merge: int = 1,
    inc_finalize: bool = False,
    seq_layout: bool = False,
    split_tail: bool = False,
) -> bass.Bass:
    # Bacc (not plain Bass): its compile() pass legalizes instructions that
    # carry multiple sync waits, which walrus rejects from raw Bass output.
    # `repeat` re-runs the whole tile loop (same data, same output) and is
    # only used for marginal-timing benchmarks; keep 1 for real use.
    nc = bacc.Bacc("TRN2", target_bir_lowering=False)
    f32 = mybir.dt.float32

    x1 = nc.dram_tensor("x1", [B_SHARD, D], f32, kind="ExternalInput")
    x2 = nc.dram_tensor("x2", [B_SHARD, D], f32, kind="ExternalInput")

    if seq_layout:
        # row = n*128 + p: every [128, D] tile is one fully-contiguous 2 MiB
        # block and the 16 tiles stream HBM perfectly sequentially. The
        # per-row results then land in out[p, n] = row n*128+p, which the
        # host unscrambles with a free transpose (see kernel()).
        out = nc.dram_tensor("out", [P, N_TILES], f32, kind="ExternalOutput")
        x1r = x1.rearrange("(n p) d -> p n d", p=P)  # [128, 16, D]
        x2r = x2.rearrange("(n p) d -> p n d", p=P)
        outr = out[:, :]  # [128, 16]
    else:
        # row = p*N_TILES + n: tile n is [128, D] with partition stride
        # N_TILES*D (16 KiB contiguous per partition, 256 KiB stride).
        out = nc.dram_tensor("out", [B_SHARD], f32, kind="ExternalOutput")
        x1r = x1.rearrange("(p n) d -> p n d", p=P)  # [128, 16, D]
        x2r = x2.rearrange("(p n) d -> p n d", p=P)
        outr = out.rearrange("(p n) -> p n", p=P)  # [128, 16]
    # With dma_merge=m, one DMA loads m consecutive n-columns ([128, m, D]);
    # compute still runs per n-column (accum_out is one scalar per row).

    with tile.TileContext(nc) as tc:
        with (
            tc.tile_pool(name="x1p", bufs=bufs) as x1p,
            tc.tile_pool(name="x2p", bufs=bufs) as x2p,
            tc.tile_pool(name="junk", bufs=1) as junkp,
            tc.tile_pool(name="stats", bufs=1) as statsp,
        ):
            sx = statsp.tile([P, N_TILES], f32)
            sy = statsp.tile([P, N_TILES], f32)
            dot = statsp.tile([P, N_TILES], f32)
            # Mandatory full-size outputs of the fused reduce ops; never read.
            junk_a = junkp.tile([P, D], f32)
            junk_v = junkp.tile([P, D], f32)

            m = dma_merge
            assert N_TILES % m == 0
            if split_tail:
                assert m == 1 and not inc_finalize
                # partial accums for the split halves of the last tile
                part = statsp.tile([P, 4], f32, name="part")

            ssx = statsp.tile([P, N_TILES], f32, name="ssx")
            ssy = statsp.tile([P, N_TILES], f32, name="ssy")
            den = statsp.tile([P, N_TILES], f32, name="den")
            rec = statsp.tile([P, N_TILES], f32, name="rec")
            res = statsp.tile([P, N_TILES], f32, name="res")

            def finalize_col(n):
                # per-column finalize while later tiles still stream in;
                # keeps only the last column's short chain in the tail
                c = slice(n, n + 1)
                nc.scalar.activation(
                    out=ssx[:, c], in_=sx[:, c],
                    func=mybir.ActivationFunctionType.Sqrt, scale=4.0,
                )
                nc.scalar.activation(
                    out=ssy[:, c], in_=sy[:, c],
                    func=mybir.ActivationFunctionType.Sqrt,
                )
                nc.vector.tensor_mul(den[:, c], ssx[:, c], ssy[:, c])
                nc.vector.reciprocal(rec[:, c], den[:, c])
                nc.vector.tensor_mul(res[:, c], dot[:, c], rec[:, c])
                # issue from the ACT HW-DGE ring: the SP ring is the dense
                # input-DMA critical path and must not carry the tiny stores
                nc.scalar.dma_start(out=outr[:, c], in_=res[:, c])

            def split_last_tile():
                # Load/compute the last tile in two half-width pieces so the
                # tail after the final byte lands is a half-width dot instead
                # of a full one (~2 us shorter kernel tail). Half sums go to
                # `part` and are combined with one tensor_add per stat.
                n = N_TILES - 1
                H = D // 2
                t1 = x1p.tile([P, D], f32, name="t1")
                t2 = x2p.tile([P, D], f32, name="t2")
                for h in (0, 1):
                    cs = slice(h * H, (h + 1) * H)
                    nc.sync.dma_start(out=t1[:, cs], in_=x1r[:, n, cs])
                    nc.sync.dma_start(out=t2[:, cs], in_=x2r[:, n, cs])
                    nc.scalar.activation(
                        out=junk_a[:, cs],
                        in_=t1[:, cs],
                        func=mybir.ActivationFunctionType.Square,
                        accum_out=(sx[:, n : n + 1] if h == 0 else part[:, 0:1]),
                    )
                    nc.scalar.activation(
                        out=junk_a[:, cs],
                        in_=t2[:, cs],
                        func=mybir.ActivationFunctionType.Square,
                        accum_out=(sy[:, n : n + 1] if h == 0 else part[:, 1:2]),
                    )
                    nc.vector.scalar_tensor_tensor(
                        out=junk_v[:, cs],
                        in0=t1[:, cs],
                        scalar=1.0,
                        in1=t2[:, cs],
                        op0=mybir.AluOpType.mult,
                        op1=mybir.AluOpType.mult,
                        accum_out=(dot[:, n : n + 1] if h == 0 else part[:, 2:3]),
                    )
                nc.vector.tensor_add(sx[:, n : n + 1], sx[:, n : n + 1], part[:, 0:1])
                nc.vector.tensor_add(sy[:, n : n + 1], sy[:, n : n + 1], part[:, 1:2])
                nc.vector.tensor_add(dot[:, n : n + 1], dot[:, n : n + 1], part[:, 2:3])

            def tile_body():
                n_groups = N_TILES // m
                if split_tail:
                    n_groups -= 1
                for g in range(n_groups):
                    n0 = g * m
                    t1 = x1p.tile([P, m, D], f32, name="t1")
                    t2 = x2p.tile([P, m, D], f32, name="t2")
                    nc.sync.dma_start(out=t1, in_=x1r[:, n0 : n0 + m, :])
                    # optionally issue x2 loads from the ACT sequencer so the
                    # two input streams use both HW-DGE rings
                    x2_eng = nc.scalar if split_rings else nc.sync
                    x2_eng.dma_start(out=t2, in_=x2r[:, n0 : n0 + m, :])
                    for j in range(m):
                        n = n0 + j
                        nc.scalar.activation(
                            out=junk_a,
                            in_=t1[:, j, :],
                            func=mybir.ActivationFunctionType.Square,
                            accum_out=sx[:, n : n + 1],
                        )
                        nc.scalar.activation(
                            out=junk_a,
                            in_=t2[:, j, :],
                            func=mybir.ActivationFunctionType.Square,
                            accum_out=sy[:, n : n + 1],
                        )
                        # Fused (t1*1.0)*t2 with accum_out = per-row sum -> dot.
                        # (tensor_tensor_reduce compiles but faults on HW; this
                        # TensorScalarPtr form is the supported fused mul+reduce.)
                        nc.vector.scalar_tensor_tensor(
                            out=junk_v,
                            in0=t1[:, j, :],
                            scalar=1.0,
                            in1=t2[:, j, :],
                            op0=mybir.AluOpType.mult,
                            op1=mybir.AluOpType.mult,
                            accum_out=dot[:, n : n + 1],
                        )
                        if inc_finalize:
                            finalize_col(n)
                if split_tail:
                    split_last_tile()

            if repeat == 1:
                tile_body()
            else:
                with tc.For_i(0, repeat, 1):
                    tile_body()

            if not inc_finalize:
                # cos/2 = dot / (2*sqrt(sx)*sqrt(sy));  sqrt(4*sx) = 2*sqrt(sx)
                nc.scalar.activation(
                    out=ssx, in_=sx, func=mybir.ActivationFunctionType.Sqrt,
                    scale=4.0,
                )
                nc.scalar.activation(
                    out=ssy, in_=sy, func=mybir.ActivationFunctionType.Sqrt
                )
                nc.vector.tensor_mul(den, ssx, ssy)
                nc.vector.reciprocal(rec, den)
                nc.vector.tensor_mul(res, dot, rec)
                nc.sync.dma_start(out=outr, in_=res)

    nc.compile()
    return nc


def kernel(x1: np.ndarray, x2: np.ndarray, **_kw) -> np.ndarray:
    global _NC_CACHE
    x1 = np.ascontiguousarray(np.asarray(x1, dtype=np.float32))
    x2 = np.ascontiguousarray(np.asarray(x2, dtype=np.float32))
    assert x1.shape == (B, D) and x2.shape == (B, D)

    in_maps = [
        {
            "x1": x1[c * B_SHARD : (c + 1) * B_SHARD],
            "x2": x2[c * B_SHARD : (c + 1) * B_SHARD],
        }
        for c in range(N_CORES)
    ]

    if _NC_CACHE is None:
        _NC_CACHE = build_kernel(seq_layout=SEQ_LAYOUT, split_tail=True)

    res = run_bass_kernel_spmd(_NC_CACHE, in_maps, core_ids=list(range(N_CORES)))
    if SEQ_LAYOUT:
        # out_core[p, n] holds shard row n*128+p -> transpose to row order
        shards = [
            np.ascontiguousarray(res.results[c]["out"].T).reshape(B_SHARD)
            for c in range(N_CORES)
        ]
    else:
        shards = [res.results[c]["out"] for c in range(N_CORES)]
    return np.concatenate(shards, axis=0)
